# revision 1
# baseline (speedup 1.0000x reference)
"""BatchHardTripletLoss on 8 Trainium2 NeuronCores.

Strategy (batch/row sharding, per the hint): core c owns anchor rows
[512c, 512c+512). All three [4096,128] tensors are passed to every core,
rolled by 512c rows so that core-local row i corresponds to global row
512c+i and the self-match diagonal sits at a *static* column block
(tile n=0, offset 128m) in every core's program (SPMD-friendly).

Per core:
  - PE transposes the three tensors to [128(d), 4096] (f32r, scaled by -2
    on the rhs copies) and squares them for y2 = ||y_j||^2 row sums.
  - Gram tiles m_ij = y2_j - 2 a_i . y_j are accumulated in PSUM:
    one K=128 f32r matmul (-2 a.y) + one K=1 ones-row matmul (+y2_j)
    (+ one K=128 bf16 BIG-identity matmul masking the diagonal for the
    anchor-anchor / anchor-pos matrices).
  - VectorE min-reduces 4-bank PSUM groups; hardest_neg^2 = a2_i + min.
  - distance_pos, sqrt, softplus, and the partial loss sum are computed
    on-chip; each core emits one scalar (sum of its 512 row losses).
Host sums the 8 partials and divides by 4096.
"""

import os
import sys

if "/opt/trn_rl_repo" not in sys.path:
    sys.path.insert(0, "/opt/trn_rl_repo")

from contextlib import ExitStack

import numpy as np

import concourse.bass as bass
import concourse.tile as tile
from concourse import bacc, bass_utils, mybir
from concourse.masks import make_identity

F32 = mybir.dt.float32
F32R = mybir.dt.float32r
BF16 = mybir.dt.bfloat16
AF = mybir.ActivationFunctionType
ALU = mybir.AluOpType

B, D, NCORES = 4096, 128, 8
RB = B // NCORES        # 512 rows per core
NCHUNK = B // 128       # 32 row chunks of 128
MT = RB // 128          # 4 m-tiles per core
NGRP = 4                # n groups per matrix (8 n-tiles of 512 / 2 per group)
GRPW = 1024             # group width (2 PSUM banks)
EPS = 1e-12
BIG = 1.0e38

_CACHE: dict = {}

# walrus emits one LDWEIGHTS per matmul with --enable-ldw-opt=false (the
# bass_utils default); flipping it lets consecutive same-weight matmuls
# skip the reload.
if os.environ.get("BASS_LDW_OPT", "0") == "1":
    if not getattr(bass_utils, "_ldw_patched", False):
        _orig_rc = bass_utils.run_command

        def _rc(argv, **kw):
            argv = ["--enable-ldw-opt=true" if a == "--enable-ldw-opt=false"
                    else a for a in argv]
            return _orig_rc(argv, **kw)

        bass_utils.run_command = _rc
        bass_utils._ldw_patched = True


def _build():
    nc = bacc.Bacc("TRN2", target_bir_lowering=False, debug=False)

    anc = nc.dram_tensor("anc", [B, D], F32, kind="ExternalInput").ap()
    pos = nc.dram_tensor("pos", [B, D], F32, kind="ExternalInput").ap()
    neg = nc.dram_tensor("neg", [B, D], F32, kind="ExternalInput").ap()
    out = nc.dram_tensor("out", [1, 1], F32, kind="ExternalOutput").ap()

    with tile.TileContext(nc) as tc:
        with ExitStack() as ctx:
            _emit(ctx, tc, nc, [anc, pos, neg], out)
    nc.compile()
    return nc


def _emit(ctx, tc, nc, ins, out_d):
    const = ctx.enter_context(tc.tile_pool(name="const", bufs=1))
    ytp = ctx.enter_context(tc.tile_pool(name="ytp", bufs=1))
    y2p = ctx.enter_context(tc.tile_pool(name="y2p", bufs=1))
    stats = ctx.enter_context(tc.tile_pool(name="stats", bufs=1))
    raw = ctx.enter_context(tc.tile_pool(name="raw", bufs=1))
    scr = ctx.enter_context(tc.tile_pool(name="scr", bufs=4))
    fin = ctx.enter_context(tc.tile_pool(name="fin", bufs=1))
    tpsum = ctx.enter_context(tc.tile_pool(name="tpsum", bufs=2, space="PSUM"))
    mpsum = ctx.enter_context(tc.tile_pool(name="mpsum", bufs=2, space="PSUM"))

    # ---- constants ----
    ident = const.tile([128, 128], F32, tag="ident",
                       name="ident_s" + os.environ.get("BASS_SALT", "0"))
    make_identity(nc, ident[:])
    eye_big = const.tile([128, 128], BF16, tag="eye_big")
    nc.scalar.activation(eye_big[:], ident[:], AF.Copy, scale=BIG)
    ibuf = const.tile([128, 1024], BF16, tag="ibuf")
    nc.vector.memset(ibuf[:, 0:512], 0.0)
    nc.vector.memset(ibuf[:, 640:1024], 0.0)
    nc.scalar.activation(ibuf[:, 512:640], ident[:], AF.Copy)
    ones_col_f = const.tile([128, 1], F32, tag="ones_col_f")
    nc.vector.memset(ones_col_f[:], 1.0)
    ones128_f = const.tile([128, 128], F32, tag="ones128_f")
    nc.vector.memset(ones128_f[:], 0.5)
    ones128 = const.tile([128, 128], BF16, tag="ones128")
    nc.scalar.activation(ones128[:], ones128_f[:], AF.Copy)

    # ---- persistent operands ----
    yt = [ytp.tile([128, B], F32R, tag=f"yt{y}", name=f"yt{y}") for y in range(3)]
    ytsq = [y2p.tile([128, B], BF16, tag=f"ytsq{y}", name=f"ytsq{y}")
            for y in range(3)]
    at2 = ytp.tile([128, RB], F32R, tag="at2")
    a2col = stats.tile([128, MT], F32, tag="a2col")
    dpsq = stats.tile([128, MT], F32, tag="dpsq")
    # group layout: per (m, y): 3 PSUM groups of n-tiles (3, 3, 2)
    GSPANS = [(0, 3), (3, 3), (6, 2)]
    mins = stats.tile([128, MT, 9], F32, tag="mins")


    # ---- input loads: anchor first, split for early transposes ----
    raws = []
    for y in range(3):
        r = raw.tile([128, NCHUNK, 128], F32, tag=f"raw{y}", name=f"raw{y}")
        raws.append(r)
    _pieces = [8, 16, 16]   # chunks per DMA: anchor lands early, rest bulk
    for y in range(3):
        rsrc = ins[y].rearrange("(n p) d -> p n d", p=128)
        np_ = _pieces[y]
        for g in range(NCHUNK // np_):
            nc.sync.dma_start(raws[y][:, np_ * g:np_ * (g + 1), :],
                              rsrc[:, np_ * g:np_ * (g + 1), :])

    # ---- row stats (small, early; only raw anc/pos chunks 0..3) ----
    for m in range(MT):
        asq = scr.tile([128, 128], F32, tag="asq")
        nc.scalar.activation(asq[:], raws[0][:, m, :], AF.Square)
        nc.vector.tensor_reduce(out=a2col[:, m:m + 1], in_=asq[:],
                                axis=mybir.AxisListType.X, op=ALU.add)
        dif = scr.tile([128, 128], F32, tag="dif")
        nc.vector.tensor_tensor(out=dif[:], in0=raws[0][:, m, :],
                                in1=raws[1][:, m, :], op=ALU.subtract)
        dsq = scr.tile([128, 128], F32, tag="dsq")
        nc.scalar.activation(dsq[:], dif[:], AF.Square)
        nc.vector.tensor_reduce(out=dpsq[:, m:m + 1], in_=dsq[:],
                                axis=mybir.AxisListType.X, op=ALU.add)

    # ---- job emission: transposes and Gram groups interleaved so the
    #      (in-order) PE queue overlaps phase A with phase B ----
    def emit_tr(y, g):
        pt = tpsum.tile([128, 512], F32, name="pt")
        for k in range(4):
            nc.tensor.transpose(pt[:, k * 128:(k + 1) * 128],
                                raws[y][:, 4 * g + k, :], ident[:])
        sl = slice(512 * g, 512 * (g + 1))
        nc.scalar.activation(yt[y][:, sl], pt[:], AF.Copy)
        # ytsq = y^2 on the idle GPSIMD (the 0.5 lives in the fold matrix)
        nc.gpsimd.tensor_tensor(out=ytsq[y][:, sl], in0=yt[y][:, sl],
                                in1=yt[y][:, sl], op=ALU.mult)
        if y == 0 and g == 0:
            nc.scalar.activation(at2[:], pt[:], AF.Copy, scale=-1.0)

    def emit_main(m, y, g):
        n0, nn = GSPANS[g]
        pg = mpsum.tile([128, nn * 512], F32, tag="mg",
                        padded_shape=[128, 1536], name="pg")
        lhs = at2[:, m * 128:(m + 1) * 128]
        masked = (g == 0 and y < 2)
        for k in range(nn):
            n = n0 + k
            nc.tensor.matmul(pg[:, k * 512:(k + 1) * 512], lhs,
                             yt[y][:, n * 512:(n + 1) * 512],
                             start=True, stop=False)
        for k in range(nn):
            n = n0 + k
            nc.tensor.matmul(pg[:, k * 512:(k + 1) * 512], ones128[:],
                             ytsq[y][:, n * 512:(n + 1) * 512],
                             start=False, stop=not (masked and k == 0))
        if masked:
            nc.tensor.matmul(pg[:, 0:512], eye_big[:],
                             ibuf[:, 512 - 128 * m:1024 - 128 * m],
                             start=False, stop=True)
        nc.vector.tensor_reduce(out=mins[:, m, 3 * g + y:3 * g + y + 1],
                                in_=pg[:], axis=mybir.AxisListType.X,
                                op=ALU.min)

    # dependency-aware emission: a main group (y, g) only needs that
    # matrix's transposes up to tile need[g]; interleave the remaining
    # transpose jobs between main groups so neither PE stream starves.
    need = [2, 5, 7]
    tr_jobs = [(y, g) for y in range(3) for g in range(8)]
    main_jobs = [(m, y, g) for y in range(3) for g in range(3)
                 for m in range(MT)]
    ti = 0
    emitted = 0
    for (m, y, g) in main_jobs:
        # ensure required transposes for this (y, g) are emitted
        while ti <= tr_jobs.index((y, need[g])):
            emit_tr(*tr_jobs[ti]); ti += 1
        emit_main(m, y, g)
        emitted += 1
        # steady drip of future transposes (1 per main group)
        if ti < len(tr_jobs) and emitted % 1 == 0:
            emit_tr(*tr_jobs[ti]); ti += 1
    while ti < len(tr_jobs):
        emit_tr(*tr_jobs[ti]); ti += 1

    # ---- final: loss ----
    hnmin = fin.tile([128, MT], F32, tag="hnmin")
    for m in range(MT):
        nc.vector.tensor_reduce(out=hnmin[:, m:m + 1], in_=mins[:, m, :],
                                axis=mybir.AxisListType.X, op=ALU.min)
    hnsq = fin.tile([128, MT], F32, tag="hnsq")
    nc.vector.tensor_scalar(out=hnsq[:], in0=hnmin[:], scalar1=2.0,
                            scalar2=None, op0=ALU.mult)
    nc.vector.tensor_tensor(out=hnsq[:], in0=hnsq[:], in1=a2col[:], op=ALU.add)
    nc.vector.tensor_scalar_max(out=hnsq[:], in0=hnsq[:], scalar1=EPS)
    nc.vector.tensor_scalar_max(out=dpsq[:], in0=dpsq[:], scalar1=EPS)
    # sqrt(x) = exp(0.5*ln(x)); Ln pair first, then Exp pair, then the
    # softplus Exp/Ln -- clusters table switches (Ln,Ln | Exp,Exp,Exp | Ln)
    hn = fin.tile([128, MT], F32, tag="hn")
    dp = fin.tile([128, MT], F32, tag="dp")
    x = fin.tile([128, MT], F32, tag="x")
    ex = fin.tile([128, MT], F32, tag="ex")
    sp = fin.tile([128, MT], F32, tag="sp")
    # explicit dep chain pins ACT order -> table switches Ln,Ln|Exp,Exp|..|Ln
    i1 = nc.scalar.activation(hn[:], hnsq[:], AF.Ln)
    i2 = nc.scalar.activation(dp[:], dpsq[:], AF.Ln)
    i3 = nc.scalar.activation(hn[:], hn[:], AF.Exp, scale=0.5)
    i4 = nc.scalar.activation(dp[:], dp[:], AF.Exp, scale=0.5)
    nc.vector.tensor_tensor(out=x[:], in0=dp[:], in1=hn[:], op=ALU.subtract)
    i5 = nc.scalar.activation(ex[:], x[:], AF.Exp)
    nc.scalar.activation(sp[:], ex[:], AF.Ln, bias=ones_col_f[:], scale=1.0)
    from concourse.bass import _add_dep_helper
    for a, b in [(i2, i1), (i3, i2), (i4, i3), (i5, i4)]:
        _add_dep_helper(a.ins, b.ins, sync=False, reason="act table order")
    lsum = fin.tile([128, 1], F32, tag="lsum")
    nc.vector.tensor_reduce(out=lsum[:], in_=sp[:],
                            axis=mybir.AxisListType.X, op=ALU.add)
    ps = tpsum.tile([1, 1], F32, tag="pt", name="ps")
    nc.tensor.matmul(ps[:], lsum[:], ones_col_f[:], start=True, stop=True)
    res = fin.tile([1, 1], F32, tag="res")
    nc.scalar.activation(res[:], ps[:], AF.Copy)
    nc.sync.dma_start(out_d, res[:])


def _get_nc():
    if "nc" not in _CACHE:
        _CACHE["nc"] = _build()
    return _CACHE["nc"]


def kernel(rep_anchor, rep_pos, rep_neg):
    A = np.ascontiguousarray(rep_anchor, dtype=np.float32)
    P = np.ascontiguousarray(rep_pos, dtype=np.float32)
    N = np.ascontiguousarray(rep_neg, dtype=np.float32)

    nc = _get_nc()
    in_maps = []
    for c in range(NCORES):
        s = RB * c
        in_maps.append({
            "anc": np.ascontiguousarray(np.concatenate([A[s:], A[:s]], axis=0)),
            "pos": np.ascontiguousarray(np.concatenate([P[s:], P[:s]], axis=0)),
            "neg": np.ascontiguousarray(np.concatenate([N[s:], N[:s]], axis=0)),
        })
    res = bass_utils.run_bass_kernel_spmd(nc, in_maps,
                                          core_ids=list(range(NCORES)))
    total = np.float64(0.0)
    for c in range(NCORES):
        total += np.float64(res.results[c]["out"][0, 0])
    return np.float32(total / B)

